# revision 92
# baseline (speedup 1.0000x reference)
"""MicroDLRM (hash-embedding DLRM) on 8 TRN2 NeuronCores.

Two SPMD passes; the host only hashes, routes indices, and permutes
payload between passes (the all-to-all of the sharding hint).

Pass 1 (row-parallel gather + data-parallel bot MLP):
  The 512MB table is row-sharded (262144 rows/core); lookups are
  bucketed per 32K-row window (8 windows, one dma_gather each, queues
  round-robined).  ACT/DVE convert each gathered window f32->bf16 in
  SBUF so the store-back is half-sized; the otherwise-idle PE runs the
  bot MLP on this core's batch shard in parallel and emits h=[8,2048]
  bf16.  vs the original baseline this merges the bot MLP into pass 1,
  halves gout traffic, and splits the idx upload so window 0's gather
  starts earlier.

Pass 2 (data-parallel top MLP): l1a/l1b/l2/l3 matmuls only (16 total,
  weight slabs stay tiny), psum double-buffered, embT chunks streamed
  per 512-column tile, sigmoid ACT table preloaded via a dummy op.
"""

import sys

sys.path.insert(0, "/opt/trn_rl_repo")

from contextlib import ExitStack

import ml_dtypes
import numpy as np

import concourse.bass as bass
import concourse.bacc as bacc
import concourse.mybir as mybir
from concourse.bass_utils import run_bass_kernel_spmd
from concourse import library_config

N_CORES = 8
V = 2_000_000
D = 64                      # embedding width
B = 16384
N_FEAT = 3
B_SH = B // N_CORES         # 2048 batch rows per core
ROWS_PER_CORE = 262144      # 8 x 262144 = 2,097,152 >= V (last core padded)
WIN_ROWS = 32768            # rows per gather window (int16-addressable)
N_WIN = ROWS_PER_CORE // WIN_ROWS       # 8
N_GQ = 4                    # SWDGE queues (ucode max; 2 measured +13us)
BF16 = mybir.dt.bfloat16
F32 = mybir.dt.float32
NT = B_SH // 512            # 4 column tiles for the MLPs

_C1 = np.int64(13787848793156543929 - (1 << 64))
_C2 = np.int64(10723151780598845931 - (1 << 64))


def _hash_mod(idx: np.ndarray, seed: np.int64, mod: int) -> np.ndarray:
    with np.errstate(over="ignore"):
        x = idx.astype(np.int64) ^ seed
        x = (x ^ (x >> np.int64(30))) * _C1
        x = (x ^ (x >> np.int64(27))) * _C2
        x = x ^ (x >> np.int64(31))
    return np.abs(x) % np.int64(mod)


# ---------------------------------------------------------------- pass 1 ----


def _build_p1_nc(caps: tuple, commons: tuple):
    """caps: per-window slot capacities (multiples of 128). commons (the
    max real count per window) is unused on-device but part of the build
    cache key."""
    nc = bacc.Bacc("TRN2", target_bir_lowering=False, num_devices=N_CORES,
                   debug=False, num_swdge_queues=N_GQ)
    nb = [c // 128 for c in caps]            # 128-slot blocks per window
    ioff = np.concatenate([[0], np.cumsum([c // 16 for c in caps])]).astype(int)
    bofs = np.concatenate([[0], np.cumsum([n * 64 for n in nb])]).astype(int)
    ICOLS, BELEM = int(ioff[-1]), int(bofs[-1])
    # gather segments (window, slot_lo, n_slots); a tail-split variant of
    # the last window measured no better (the extra instruction's fixed Q7
    # generation cost cancels the smaller final drain), so 1 seg = 1 window
    segs = [(w, 0, caps[w]) for w in range(N_WIN)]
    N_SEG = len(segs)

    def seg_icols(s):
        w, lo, n = segs[s]
        return int(ioff[w] + lo // 16), int(ioff[w] + (lo + n) // 16)

    def seg_elems(s):
        w, lo, n = segs[s]
        base = int(bofs[w] + (lo // 128) * 64)
        return base, base + (n // 128) * 64

    table = nc.declare_dram_parameter(
        "table", [ROWS_PER_CORE, D], F32, isOutput=False)
    idx = nc.declare_dram_parameter(
        "idx", [128, ICOLS], mybir.dt.int16, isOutput=False)
    xt = nc.declare_dram_parameter("xt", [16, B_SH], BF16, isOutput=False)
    wb = nc.declare_dram_parameter("wb", [16, 16], BF16, isOutput=False)
    bb = nc.declare_dram_parameter("bb", [8, 2], F32, isOutput=False)
    gout = nc.declare_dram_parameter("gout", [128, BELEM], BF16, isOutput=True)
    hout = nc.declare_dram_parameter("hout", [8, B_SH], BF16, isOutput=True)

    with ExitStack() as ctx:
        e = ctx.enter_context
        idx_s = e(nc.sbuf_tensor([128, ICOLS], mybir.dt.int16))
        i_sem1 = e(nc.semaphore("i_sem1"))
        gt = e(nc.sbuf_tensor([128, BELEM], F32))
        gtb = e(nc.sbuf_tensor([128, BELEM], BF16))
        xt_s = e(nc.sbuf_tensor([16, B_SH], BF16))
        h0 = e(nc.sbuf_tensor([8, B_SH], BF16))
        h = e(nc.sbuf_tensor([8, B_SH], BF16))
        ws = e(nc.sbuf_tensor([16, 16], BF16))
        bs = e(nc.sbuf_tensor([8, 2], F32))
        pb0 = [e(nc.psum_tensor(f"pb0{i}", [128, 512], F32)) for i in (0, 1)]
        pb1 = [e(nc.psum_tensor(f"pb1{i}", [128, 512], F32)) for i in (0, 1)]
        i_sem = e(nc.semaphore("i_sem"))
        g_sems = [e(nc.semaphore(f"g_sem{s}")) for s in range(N_SEG)]
        c_sems = [e(nc.semaphore(f"c_sem{s}")) for s in range(N_SEG)]
        s_sem = e(nc.semaphore("s_sem"))
        d_sem = e(nc.semaphore("d_sem"))
        mm_sem = e(nc.semaphore("mm_sem"))
        as_sem = e(nc.semaphore("as_sem"))   # a1 relu on scalar
        vs_sem = e(nc.semaphore("vs_sem"))   # a2 on vector
        block = e(nc.Block())
        tab_ap = table.ap()

        def gt_view(s):
            lo, hi = seg_elems(s)
            a = gt.ap()
            return bass.AP(a.tensor, a.offset + lo,
                           [a.ap[0], [D, (hi - lo) // 64], [1, D]])

        C = [slice(t * 512, (t + 1) * 512) for t in range(NT)]
        wb0 = ws[0:16, 0:8]
        wb1 = ws[0:8, 8:16]
        bb0 = bs[0:8, 0:1]
        bb1 = bs[0:8, 1:2]
        # PE order: b0_0 b0_1 b1_0 b0_2 b1_1 b0_3 b1_2 b1_3
        pe_order = [('b0', 0), ('b0', 1), ('b1', 0), ('b0', 2),
                    ('b1', 1), ('b0', 3), ('b1', 2), ('b1', 3)]
        mm_pos = {op: i + 1 for i, op in enumerate(pe_order)}
        as_pos = {t: i + 1 for i, t in enumerate(range(NT))}   # a1 t
        vs_pos = {t: i + 1 for i, t in enumerate(range(NT))}   # a2 t

        @block.gpsimd
        def _(gpsimd):
            gpsimd.load_library(library_config.mlp)
            gpsimd.wait_ge(i_sem, 16)
            for s in range(N_SEG):
                if s == 1:
                    gpsimd.wait_ge(i_sem1, 16)
                w, _lo, n = segs[s]
                ic0, ic1 = seg_icols(s)
                gpsimd.dma_gather(
                    out_ap=gt_view(s),
                    in_ap=tab_ap[w * WIN_ROWS:(w + 1) * WIN_ROWS, :],
                    idxs_ap=idx_s[:, ic0:ic1],
                    num_idxs=n, num_idxs_reg=n,
                    elem_size=D, queue_num=s % N_GQ,
                ).then_inc(g_sems[s], 16)

        @block.sync
        def _(sync):
            sync.dma_start(out=idx_s[:, 0:int(ioff[1])],
                           in_=idx.ap()[:, 0:int(ioff[1])]).then_inc(i_sem, 16)
            sync.dma_start(out=idx_s[:, int(ioff[1]):ICOLS],
                           in_=idx.ap()[:, int(ioff[1]):ICOLS]
                           ).then_inc(i_sem1, 16)
            sync.wait_ge(vs_sem, vs_pos[NT - 1])
            sync.dma_start(out=hout.ap()[:], in_=h[:]).then_inc(s_sem, 16)
            for s in range(N_SEG):
                lo, hi = seg_elems(s)
                sync.wait_ge(c_sems[s], 1)
                sync.dma_start(out=gout.ap()[:, lo:hi],
                               in_=gtb[:, lo:hi]).then_inc(s_sem, 16)
            sync.wait_ge(s_sem, 16 * (N_SEG + 1))

        def cvt(eng, s):
            eng.wait_ge(g_sems[s], 16)
            lo, hi = seg_elems(s)
            if hasattr(eng, "tensor_scalar"):
                ins = eng.tensor_scalar(gtb[:, lo:hi], gt[:, lo:hi], 0.0,
                                        None, mybir.AluOpType.add)
            else:
                ins = eng.activation(gtb[:, lo:hi], gt[:, lo:hi],
                                     mybir.ActivationFunctionType.Copy)
            ins.then_inc(c_sems[s], 1)

        @block.tensor
        def _(tensor):
            tensor.wait_ge(d_sem, 48)
            for op in pe_order:
                kind, t = op
                if kind == 'b0':
                    if t >= 2:
                        tensor.wait_ge(as_sem, as_pos[t - 2])
                    tensor.matmul(pb0[t % 2].ap()[0:8, :], wb0, xt_s[:, C[t]],
                                  start=True, stop=True).then_inc(mm_sem)
                else:
                    tensor.wait_ge(as_sem, as_pos[t])
                    if t >= 2:
                        tensor.wait_ge(vs_sem, vs_pos[t - 2])
                    tensor.matmul(pb1[t % 2].ap()[0:8, :], wb1, h0[:, C[t]],
                                  start=True, stop=True).then_inc(mm_sem)

        @block.scalar
        def _(scalar):
            Relu = mybir.ActivationFunctionType.Relu
            scalar.dma_start(out=xt_s[:], in_=xt[:]).then_inc(d_sem, 16)
            scalar.dma_start(out=ws[:], in_=wb[:]).then_inc(d_sem, 16)
            scalar.dma_start(out=bs[:], in_=bb[:]).then_inc(d_sem, 16)
            order = ([('a1', 0), ('a1', 1), ('cv', 0), ('a1', 2), ('cv', 2),
                      ('a1', 3), ('cv', 4), ('cv', 6)]
                     + [('cv', s) for s in range(8, N_SEG)])
            for kind, t in order:
                if kind == 'a1':
                    scalar.wait_ge(mm_sem, mm_pos[('b0', t)])
                    scalar.activation(h0[:, C[t]], pb0[t % 2].ap()[0:8, :],
                                      Relu, bias=bb0).then_inc(as_sem)
                else:
                    cvt(scalar, t)

        @block.vector
        def _(vector):
            add = mybir.AluOpType.add
            mx = mybir.AluOpType.max
            order = [('a2', 0), ('a2', 1), ('cv', 1), ('a2', 2), ('cv', 3),
                     ('a2', 3), ('cv', 5), ('cv', 7)]
            for kind, t in order:
                if kind == 'a2':
                    vector.wait_ge(mm_sem, mm_pos[('b1', t)])
                    vector.tensor_scalar(h[0:8, C[t]], pb1[t % 2].ap()[0:8, :],
                                         bb1, 0.0, add, mx).then_inc(vs_sem)
                else:
                    cvt(vector, t)

    nc.compile()
    return nc


# ---------------------------------------------------------------- pass 2 ----


def _build_p2_nc():
    """Top MLP only: z=[h(8); e0; e1; e2] (200 rows) -> 32 -> 16 -> 1."""
    nc = bacc.Bacc("TRN2", target_bir_lowering=False, num_devices=N_CORES,
                   debug=False)
    embT = nc.declare_dram_parameter(
        "embT", [N_FEAT * D, B_SH], BF16, isOutput=False)
    h = nc.declare_dram_parameter("h", [8, B_SH], BF16, isOutput=False)
    wslab = nc.declare_dram_parameter("wslab", [128, 81], BF16, isOutput=False)
    bslab = nc.declare_dram_parameter("bslab", [32, 3], F32, isOutput=False)
    out = nc.declare_dram_parameter("out", [1, B_SH], F32, isOutput=True)

    with ExitStack() as ctx:
        e = ctx.enter_context
        zA = e(nc.sbuf_tensor([128, B_SH], BF16))   # 0:8 h, 8:128 embT[0:120]
        zB = e(nc.sbuf_tensor([72, B_SH], BF16))    # embT[120:192]
        t1o = e(nc.sbuf_tensor([32, B_SH], BF16))
        t2o = e(nc.sbuf_tensor([16, B_SH], BF16))
        o_s = e(nc.sbuf_tensor([1, B_SH], F32))
        ws = e(nc.sbuf_tensor([128, 81], BF16))
        bs = e(nc.sbuf_tensor([32, 3], F32))
        scr = e(nc.sbuf_tensor([1, 3], F32))
        dum = e(nc.sbuf_tensor([128, 512], BF16))
        pl1 = [e(nc.psum_tensor(f"pl1{i}", [128, 512], F32)) for i in (0, 1)]
        pl2 = [e(nc.psum_tensor(f"pl2{i}", [128, 512], F32)) for i in (0, 1)]
        pl3 = [e(nc.psum_tensor(f"pl3{i}", [128, 512], F32)) for i in (0, 1)]
        pdum = e(nc.psum_tensor("pdum", [128, 512], F32))
        d_sem = e(nc.semaphore("d_sem"))
        z_sem = e(nc.semaphore("z_sem"))
        wu_sem = e(nc.semaphore("wu_sem"))
        eA_sems = [e(nc.semaphore(f"eA_sem{t}")) for t in range(NT)]
        eB_sems = [e(nc.semaphore(f"eB_sem{t}")) for t in range(NT)]
        mm_sem = e(nc.semaphore("mm_sem"))
        as_sem = e(nc.semaphore("as_sem"))
        vs_sem = e(nc.semaphore("vs_sem"))
        o_sem = e(nc.semaphore("o_sem"))
        block = e(nc.Block())

        tw0a = ws[0:128, 0:32]
        tw0b = ws[0:72, 32:64]
        tw1 = ws[0:32, 64:80]
        tw2 = ws[0:16, 80:81]
        tb0 = bs[0:32, 0:1]
        tb1 = bs[0:16, 1:2]
        tb2 = bs[0:1, 2:3]
        C = [slice(t * 512, (t + 1) * 512) for t in range(NT)]

        pe_order = [('l1a', 0), ('l1b', 0), ('l1a', 1), ('l1b', 1),
                    ('l2', 0), ('l1a', 2), ('l1b', 2), ('l2', 1),
                    ('l3', 0), ('l1a', 3), ('l1b', 3), ('l2', 2),
                    ('l3', 1), ('l2', 3), ('l3', 2), ('l3', 3)]
        mm_pos = {op: i + 1 for i, op in enumerate(pe_order)}
        act_order = [('a3', 0), ('a3', 1), ('a5', 0), ('a3', 2),
                     ('a3', 3), ('a5', 1), ('a5', 2), ('a5', 3)]
        as_pos = {op: i + 1 for i, op in enumerate(act_order)}
        vs_pos = {t: t + 1 for t in range(NT)}

        @block.sync
        def _(sync):
            sync.dma_start(out=zA[0:8, :], in_=h[:]).then_inc(d_sem, 16)
            for t in range(NT):
                sync.dma_start(out=zA[8:128, C[t]],
                               in_=embT[0:120, C[t]]).then_inc(eA_sems[t], 16)
                sync.dma_start(out=zB[:, C[t]],
                               in_=embT[120:192, C[t]]).then_inc(eB_sems[t], 16)
            sync.wait_ge(as_sem, as_pos[('a5', NT - 1)])
            sync.dma_start(out=out.ap()[:], in_=o_s[:]).then_inc(o_sem, 16)
            sync.wait_ge(o_sem, 16)

        @block.tensor
        def _(tensor):
            # warm the PE p-state with throwaway matmuls while inputs fly in
            # (dedicated zeroed buffer; pl3[0] rows 1: are never read)
            tensor.wait_ge(wu_sem, 1)
            for _ in range(8):
                tensor.matmul(pl3[0].ap()[0:128, :], dum[:, 0:128], dum[:],
                              start=True, stop=True)
            tensor.wait_ge(d_sem, 48)
            for op in pe_order:
                kind, t = op
                if kind == 'l1a':
                    tensor.wait_ge(eA_sems[t], 16)
                    if t >= 2:
                        tensor.wait_ge(as_sem, as_pos[('a3', t - 2)])
                    tensor.matmul(pl1[t % 2].ap()[0:32, :], tw0a, zA[:, C[t]],
                                  start=True, stop=False).then_inc(mm_sem)
                elif kind == 'l1b':
                    tensor.wait_ge(eB_sems[t], 16)
                    tensor.matmul(pl1[t % 2].ap()[0:32, :], tw0b, zB[:, C[t]],
                                  start=False, stop=True).then_inc(mm_sem)
                elif kind == 'l2':
                    tensor.wait_ge(as_sem, as_pos[('a3', t)])
                    if t >= 2:
                        tensor.wait_ge(vs_sem, vs_pos[t - 2])
                    tensor.matmul(pl2[t % 2].ap()[0:16, :], tw1, t1o[:, C[t]],
                                  start=True, stop=True).then_inc(mm_sem)
                else:  # l3
                    tensor.wait_ge(vs_sem, vs_pos[t])
                    if t >= 2:
                        tensor.wait_ge(as_sem, as_pos[('a5', t - 2)])
                    tensor.matmul(pl3[t % 2].ap()[0:1, :], tw2, t2o[:, C[t]],
                                  start=True, stop=True).then_inc(mm_sem)

        @block.scalar
        def _(scalar):
            Relu = mybir.ActivationFunctionType.Relu
            Sigmoid = mybir.ActivationFunctionType.Sigmoid
            scalar.dma_start(out=ws[:], in_=wslab[:]).then_inc(d_sem, 16)
            scalar.dma_start(out=bs[:], in_=bslab[:]).then_inc(d_sem, 16)
            # preload both ACT tables while the DMAs fly (outputs unread)
            scalar.wait_ge(z_sem, 1)
            scalar.activation(scr[0:1, 1:2], scr[0:1, 0:1], Sigmoid)
            scalar.activation(scr[0:1, 2:3], scr[0:1, 0:1], Relu)
            for kind, t in act_order:
                if kind == 'a3':
                    scalar.wait_ge(mm_sem, mm_pos[('l1b', t)])
                    scalar.activation(t1o[:, C[t]], pl1[t % 2].ap()[0:32, :],
                                      Relu, bias=tb0).then_inc(as_sem)
                else:  # a5
                    scalar.wait_ge(mm_sem, mm_pos[('l3', t)])
                    scalar.activation(o_s[:, C[t]], pl3[t % 2].ap()[0:1, :],
                                      Sigmoid, bias=tb2).then_inc(as_sem)

        @block.vector
        def _(vector):
            add = mybir.AluOpType.add
            mx = mybir.AluOpType.max
            vector.memset(scr[:], 0.0).then_inc(z_sem, 1)
            vector.memset(dum[:], 0.0).then_inc(wu_sem, 1)
            for t in range(NT):
                vector.wait_ge(mm_sem, mm_pos[('l2', t)])
                vector.tensor_scalar(t2o[:, C[t]], pl2[t % 2].ap()[0:16, :],
                                     tb1, 0.0, add, mx).then_inc(vs_sem)

    nc.compile()
    return nc


# ------------------------------------------------------------------ host ----


def _route(rows: np.ndarray):
    """Bucket hashed rows by (owning core, window); slots sorted by row id."""
    flat = rows.reshape(-1)                       # j = i*B + b
    core = flat // ROWS_PER_CORE
    loc = flat - core * ROWS_PER_CORE
    w = loc // WIN_ROWS                           # window (32768 rows)
    li = (loc % WIN_ROWS).astype(np.int64)        # window-local row

    run = core * N_WIN + w
    order = np.argsort(run * WIN_ROWS + li, kind="stable")
    sorted_run = run[order]
    run_start = np.searchsorted(sorted_run, np.arange(N_CORES * N_WIN))
    rank_sorted = np.arange(flat.size) - run_start[sorted_run]
    jwin = np.empty(flat.size, np.int64)          # slot within window
    jwin[order] = rank_sorted
    counts = np.bincount(run, minlength=N_CORES * N_WIN).reshape(
        N_CORES, N_WIN)
    caps = tuple(int(max(1, -(-int(counts[:, v].max()) // 128))) * 128
                 for v in range(N_WIN))
    nb = np.array([c // 128 for c in caps])
    ioff = np.concatenate([[0], np.cumsum([c // 16 for c in caps])])
    bofs = np.concatenate([[0], np.cumsum(nb * 64)])
    ICOLS = int(ioff[-1])

    icol = (ioff[w] + jwin // 16).astype(np.int64)
    iprt = (jwin % 16).astype(np.int64)

    # idx tensor; pads read spread junk rows so every core runs an identical
    # full-capacity descriptor stream (-1 tail-trim variants measured slower
    # and reg < live-count crashes the runtime)
    commons = tuple(int(counts[:, v].max()) for v in range(N_WIN))
    idx_flat = np.empty((N_CORES, 16, ICOLS), np.int16)
    for v in range(N_WIN):
        k = caps[v]
        spread = ((np.arange(k, dtype=np.int64) * 2287) % WIN_ROWS).astype(
            np.int16)
        blk = spread.reshape(k // 16, 16).T
        idx_flat[:, :, int(ioff[v]):int(ioff[v + 1])] = blk[None]
    idx_flat[core, iprt, icol] = li.astype(np.int16)
    idx_in = np.ascontiguousarray(
        np.broadcast_to(idx_flat[:, None], (N_CORES, 8, 16, ICOLS))
        .reshape(N_CORES, 128, ICOLS))
    return core, w, jwin, caps, commons, bofs, idx_in


_CACHE = {}
TRACE = False          # set True (with BASS_PERFETTO_PROFILE_ALL_CORES=1)
LAST_EXEC_NS = {}      # pass name -> exec_time_ns of slowest core


def _get_nc(key, builder, *args):
    if key not in _CACHE:
        _CACHE[key] = builder(*args)
    return _CACHE[key]


def kernel(dense_x, sparse_idx, sparse_offsets, hash_seeds, emb_table,
           bot_w0, bot_b0, bot_w1, bot_b1,
           top_w0, top_b0, top_w1, top_b1, top_w2, top_b2):
    dense_x = np.asarray(dense_x, np.float32)
    sparse_idx = np.asarray(sparse_idx, np.int64)
    offs = np.asarray(sparse_offsets, np.int64)
    if not np.array_equal(offs, np.arange(B, dtype=np.int64)):
        raise NotImplementedError("kernel assumes one-index bags "
                                  "(sparse_offsets == arange(B))")
    hash_seeds = np.asarray(hash_seeds, np.int64)
    emb_table = np.asarray(emb_table, np.float32)
    bf16 = ml_dtypes.bfloat16

    # ---- hash + route (host: routing metadata only)
    rows = np.empty((N_FEAT, B), np.int64)
    for i in range(N_FEAT):
        rows[i] = _hash_mod(sparse_idx[i], hash_seeds[i], V)
    core, w, jwin, caps, commons, bofs, idx_in = _route(rows)

    pad_rows = N_CORES * ROWS_PER_CORE - V
    table_pad = np.concatenate(
        [emb_table, np.zeros((pad_rows, D), np.float32)], axis=0)
    table_sh = table_pad.reshape(N_CORES, ROWS_PER_CORE, D)

    xt_all = np.ascontiguousarray(
        dense_x.T.reshape(16, N_CORES, B_SH).transpose(1, 0, 2)).astype(bf16)
    wbs = np.zeros((16, 16), bf16)
    wbs[0:16, 0:8] = np.asarray(bot_w0, np.float32).astype(bf16)
    wbs[0:8, 8:16] = np.asarray(bot_w1, np.float32).astype(bf16)
    bbs = np.zeros((8, 2), np.float32)
    bbs[:, 0] = np.asarray(bot_b0, np.float32)
    bbs[:, 1] = np.asarray(bot_b1, np.float32)

    nc1 = _get_nc(("p1", caps, commons), _build_p1_nc, caps, commons)
    in_maps1 = [
        {"table": table_sh[c], "idx": idx_in[c], "xt": xt_all[c],
         "wb": wbs, "bb": bbs}
        for c in range(N_CORES)
    ]
    r1 = run_bass_kernel_spmd(nc1, in_maps1, list(range(N_CORES)), trace=TRACE)
    LAST_EXEC_NS["gather"] = r1.exec_time_ns
    res1 = r1.results

    # ---- host all-to-all: gout slots -> embT [core2, (feat, dim), batch]
    G = np.stack([res1[c]["gout"] for c in range(N_CORES)])  # [8,128,BELEM]
    p = (jwin % 128).astype(np.int64)
    colbase = (bofs[w] + (jwin // 128) * 64).astype(np.int64)
    vals = G[core[:, None], p[:, None], colbase[:, None] + np.arange(64)]
    embT_all = np.ascontiguousarray(
        vals.reshape(N_FEAT, N_CORES, B_SH, D)
        .transpose(1, 0, 3, 2).reshape(N_CORES, N_FEAT * D, B_SH))

    wslab = np.zeros((128, 81), bf16)
    tw0f = np.asarray(top_w0, np.float32).astype(bf16)
    wslab[0:128, 0:32] = tw0f[0:128]
    wslab[0:72, 32:64] = tw0f[128:200]
    wslab[0:32, 64:80] = np.asarray(top_w1, np.float32).astype(bf16)
    wslab[0:16, 80:81] = np.asarray(top_w2, np.float32).astype(bf16)
    bslab = np.zeros((32, 3), np.float32)
    bslab[0:32, 0] = np.asarray(top_b0, np.float32)
    bslab[0:16, 1] = np.asarray(top_b1, np.float32)
    bslab[0:1, 2] = np.asarray(top_b2, np.float32)

    nc2 = _get_nc(("p2",), _build_p2_nc)
    in_maps2 = [
        {"embT": embT_all[c], "h": res1[c]["hout"],
         "wslab": wslab, "bslab": bslab}
        for c in range(N_CORES)
    ]
    r2 = run_bass_kernel_spmd(nc2, in_maps2, list(range(N_CORES)), trace=TRACE)
    LAST_EXEC_NS["mlp"] = r2.exec_time_ns
    res2 = r2.results

    out = np.concatenate([res2[c]["out"][0] for c in range(N_CORES)])
    return out.reshape(B, 1).astype(np.float64)
